# revision 1
# baseline (speedup 1.0000x reference)
"""Social-GAN style decoder (nn_Decoder_85066122265358).

The reference's per-scene [P,P,P] adjacency einsums are collapsed with a
closed form exploiting adj_all[i,j,k] = (j==k)|(i==j)|(i==k):
  - for i != j only k in {i, j} survive the mask row,
  - for i == j the row is all-ones (a mask-weighted mean over the group).
With the reference's swapped-denominator normalization this yields, per mask
m (same/diff), per scene:
  agg(H)[i,j] = (m[i,j]*H[i,i] + H[i,j]) / (1 + m[i,j])      (i != j)
  agg(H)[i,i] = sum_k m[i,k]*H[i,k] / cnt_i
turning the O(P^3) einsum into O(P^2) gathers — validated to 5.6e-8 max
relative error against a literal port of the reference.

Layer 1 is further collapsed: X[i,k] = [ (pos_k - pos_i) @ W_pse + b_pse,
hid_k ], so X[i,k] @ W1 = t[k] - u[i] with u = pos @ (W_pse @ W1_top) and
t = u + hid @ W1_bot + b_pse @ W1_top — no [P,P,ED] embedding tensor is ever
built.

Computation is blocked per scene so all temporaries ([P,P,GH] = 2.4 MB) stay
cache-resident.
"""

import numpy as np

S, P, HD, ED, GH, GO, MD, T = 128, 32, 32, 16, 72, 8, 64, 8
B = S * P


def _sigmoid(x):
    out = np.empty_like(x)
    np.negative(x, out=out)
    np.exp(out, out=out)
    out += 1.0
    np.reciprocal(out, out=out)
    return out


def kernel(**inputs):
    f32 = np.float32
    inp = {k: np.asarray(v) for k, v in inputs.items()}
    g = inp["end_group"].reshape(S, P)
    W = {k: v.astype(f32) if v.dtype != f32 else v for k, v in inp.items()
         if k not in ("seq_start_end", "end_group")}

    eye = np.eye(P, dtype=bool)[None]
    same = ((g[:, :, None] == g[:, None, :]) & (g[:, :, None] != 0)) | eye
    diff = (~same) | eye

    packs = []
    for m, W1, W2 in ((same, W["W1a"], W["W2a"]), (diff, W["W1b"], W["W2b"])):
        mf = m.astype(f32)
        mo = mf * (1.0 - np.eye(P, dtype=f32))     # mask with diag zeroed
        invp = 1.0 / (1.0 + mo)                    # [S,P,P] pair denom
        moi = mo * invp
        cnt = mf.sum(-1)                           # [S,P] incl diag
        wnorm = mf / cnt[:, :, None]               # diag-row avg weights
        Wf = W["W_pse"] @ W1[:ED]                  # [2,GH]
        bf = W["b_pse"] @ W1[:ED]                  # [GH]
        packs.append((invp, moi, wnorm, Wf, bf,
                      np.ascontiguousarray(W1[ED:]), np.ascontiguousarray(W2)))

    ii = np.arange(P)
    WihT = np.ascontiguousarray(W["Wih"].T)
    WhhT = np.ascontiguousarray(W["Whh"].T)

    h = W["hh"].copy()
    c = W["ch"].copy()
    lp = W["last_pos"].copy()
    x = W["last_pos_rel"] @ W["W_se"] + W["b_se"]
    rels = np.empty((T, B, 2), f32)
    pre = np.empty((P, P, GH), f32)

    for step in range(T):
        gates = x @ WihT + W["bih"] + h @ WhhT + W["bhh"]
        ig, fg, gg, og = np.split(gates, 4, axis=-1)
        c = _sigmoid(fg) * c + _sigmoid(ig) * np.tanh(gg)
        h2 = _sigmoid(og) * np.tanh(c)
        rel_pos = h2 @ W["W_hp"] + W["b_hp"]
        cur = rel_pos + lp

        hs = h2.reshape(S, P, HD)
        ps = cur.reshape(S, P, 2)
        phs = []
        for invp, moi, wnorm, Wf, bf, W1bot, W2 in packs:
            u_all = ps @ Wf                        # [S,P,GH]
            t_all = u_all + hs @ W1bot + bf
            out = np.empty((S, P, GO), f32)
            for s in range(S):
                t, u = t_all[s], u_all[s]
                # pre[i,j] = invp[i,j]*t[j] + moi[i,j]*t[i] - u[i]; diag -> tbar[i]-u[i]
                np.multiply(invp[s][:, :, None], t[None, :, :], out=pre)
                pre += moi[s][:, :, None] * t[:, None, :]
                pre[ii, ii, :] = wnorm[s] @ t
                pre -= u[:, None, :]
                np.maximum(pre, 0.0, out=pre)      # H1
                G = (pre.reshape(P * P, GH) @ W2).reshape(P, P, GO)
                Gd = G[ii, ii, :]
                o2 = invp[s][:, :, None] * G
                o2 += moi[s][:, :, None] * Gd[:, None, :]
                o2[ii, ii, :] = (wnorm[s][:, :, None] * G).sum(1)
                np.maximum(o2, 0.0, out=o2)
                out[s] = o2.max(1)
            phs.append(out.reshape(B, GO))

        ph = np.concatenate(phs, -1)
        dh = np.maximum(np.concatenate([h2, ph], -1) @ W["W_m1"] + W["b_m1"], 0)
        h = np.maximum(dh @ W["W_m2"] + W["b_m2"], 0)
        lp = cur
        x = rel_pos @ W["W_se"] + W["b_se"]
        rels[step] = rel_pos
    return rels



# revision 14
# speedup vs baseline: 7.3171x; 7.3171x over previous
"""Social-GAN decoder (nn_Decoder_85066122265358) on 8 Trainium2 cores.

Math (validated against the reference to 5.6e-8 in the numpy port):
the per-scene [P,P,P] adjacency einsums collapse, via
adj_all[i,j,k] = (j==k)|(i==j)|(i==k) and the reference's
swapped-denominator normalize, to

  pre[i,j] = invp[i,j]*t[j] + moi[i,j]*t[i] - u[i]   (i != j)
  pre[i,i] = wnorm[i,:] @ t - u[i]
  o2[i,j]  = relu(invp[i,j]*G[i,j] + moi[i,j]*G[i,i])  (i != j)
  o2[i,i]  = relu(wnorm[i,:] . G[i,:])
  ph[i]    = max_j o2[i,j]

with t = pos@Wf + hid@W1bot + bf, u = pos@Wf, G = relu(pre)@W2,
invp = 1/(1+mo), moi = mo*invp, mo the zero-diag mask, wnorm = mask/cnt.

Device mapping: everything is kept in transposed [feature, ped] layout so
the LSTM/MLP/pooling chain is pure TensorEngine work:
  - layer-1 pre^T [72, P*P] per (scene,mask) is ONE matmul
    lhsT=[t;u] [64,72] bf16 x C [64, 1024] bf16, where C bakes the
    invp/moi/wnorm/-1 coefficients (constant across time steps).
  - layer-2 G^T [8, P*P] = W2^T @ relu(pre^T), again matmuls.
  - the o2 aggregation runs on the Vector engine over [128, 1024] blocks
    (16 (scene,mask) groups of 8 partitions), using 8x-replicated
    invp/moi/wnorm tiles (constant, uploaded once).
Scenes are data-parallel: 16 scenes per core, no collectives.

Constants (C matrices, masks, weights) are uploaded once and cached on
device; steady-state calls transfer only the [*, 512] state slices.
"""

import hashlib

import numpy as np
import ml_dtypes

S, P, HD, ED, GH, GO, MD, T = 128, 32, 32, 16, 72, 8, 64, 8
B = S * P
NCORES = 8
SC = S // NCORES          # scenes per core
BC = SC * P               # peds per core
NBLK = SC // 8            # agg2 blocks (8 scenes x 2 masks = 16 groups) per core

_RT = None                # (sharded_fn, in_names, n_params, out_names, out_shapes)
_CONSTS = None            # (key, dict name -> jax device array)


# ---------------------------------------------------------------- host prep

def _mask_stats(end_group):
    g = np.asarray(end_group).reshape(S, P)
    eye = np.eye(P, dtype=bool)
    same = ((g[:, :, None] == g[:, None, :]) & (g[:, :, None] != 0)) | eye
    diff = (~same) | eye
    m = np.stack([same, diff], 1).astype(np.float32)      # [S,2,P,P]
    mo = m * (1.0 - np.eye(P, dtype=np.float32))
    invp = 1.0 / (1.0 + mo)
    moi = mo * invp
    wnorm = m / m.sum(-1, keepdims=True)
    return invp, moi, wnorm


def _build_C(invp, moi, wnorm):
    """C[s, m, k, i*P+j]: pre^T = [t;u]^T @ C  (see module docstring)."""
    C = np.zeros((S, 2, 64, P, P), np.float32)
    for k in range(P):
        C[:, :, k, :, k] += invp[:, :, :, k]
        C[:, :, k, k, :] += moi[:, :, k, :]
        C[:, :, 32 + k, k, :] = -1.0
    ii = np.arange(P)
    kk = np.repeat(np.arange(P), P).reshape(P, P)          # [k, i] -> k
    iw = np.tile(ii, (P, 1))                               # [k, i] -> i
    C[:, :, kk, iw, iw] = wnorm[:, :, iw, kk]
    return C.reshape(S, 2, 64, P * P)


def _prep_consts(inp):
    f32 = np.float32
    W = {k: np.asarray(v, f32) for k, v in inp.items()
         if k not in ("seq_start_end", "end_group")}
    invp, moi, wnorm = _mask_stats(inp["end_group"])
    C = _build_C(invp, moi, wnorm)

    def rep8(a):  # [S,2,P,P] -> per-core [128, NBLK*1024], p = ls*16+m*8+o
        a = a.reshape(NCORES, NBLK, 8, 2, P, P)
        a = np.broadcast_to(a[:, :, :, :, None], (NCORES, NBLK, 8, 2, 8, P, P))
        a = a.reshape(NCORES, NBLK, 128, P * P).transpose(0, 2, 1, 3)
        return np.ascontiguousarray(a.reshape(NCORES, 128, NBLK * P * P))

    out = {}
    out["invp8"], out["moi8"], out["wnorm8"] = rep8(invp), rep8(moi), rep8(wnorm)
    Cc = C.reshape(NCORES, SC * 2, 64, P * P).transpose(0, 2, 1, 3)
    out["Cmat"] = np.ascontiguousarray(
        Cc.reshape(NCORES, 64, SC * 2 * P * P)).astype(ml_dtypes.bfloat16)

    def rep(a):  # replicate weight across cores
        return np.broadcast_to(a, (NCORES,) + a.shape)

    out["WihT"] = rep(W["Wih"].T.copy())                   # [16,128]
    out["WhhT"] = rep(W["Whh"].T.copy())                   # [32,128]
    out["bg"] = rep((W["bih"] + W["bhh"])[:, None])        # [128,1]
    out["Whp"] = rep(W["W_hp"])                            # [32,2]
    out["bhp"] = rep(W["b_hp"][:, None])                   # [2,1]
    out["Wse"] = rep(W["W_se"])                            # [2,16]
    out["bse"] = rep(W["b_se"][:, None])                   # [16,1]

    # U row order: [h2(0:32); pos(32:34); ones(34); ph(35:51)]
    Wst = np.zeros((35, 288), f32)           # t cols 0:144, u cols 144:288
    W2p = np.zeros((16, 72, 128), f32)       # G^T compaction weights
    for m, (W1, W2) in enumerate(((W["W1a"], W["W2a"]), (W["W1b"], W["W2b"]))):
        Wf = W["W_pse"] @ W1[:ED]
        Wst[0:32, m * 72:(m + 1) * 72] = W1[ED:]
        Wst[32:34, m * 72:(m + 1) * 72] = Wf
        Wst[34, m * 72:(m + 1) * 72] = W["b_pse"] @ W1[:ED]
        Wst[32:34, 144 + m * 72:144 + (m + 1) * 72] = Wf
        for ls in range(8):
            goff = ls * 16 + m * 8
            W2p[ls * 2 + m, :, goff:goff + 8] = W2
    out["Wst"] = rep(Wst)
    out["W2p"] = rep(W2p.transpose(1, 0, 2).reshape(72, 16 * 128)
                     .astype(ml_dtypes.bfloat16))

    Wm1 = np.concatenate([W["W_m1"][0:32], np.zeros((2, MD), f32),
                          W["b_m1"][None], W["W_m1"][32:48]], 0)
    Wm2 = np.concatenate([W["W_m2"], W["b_m2"][None]], 0)
    out["Wm1"], out["Wm2"] = rep(Wm1), rep(Wm2)
    return {k: np.ascontiguousarray(v) for k, v in out.items()}


def _prep_state(inp):
    f32 = np.float32
    lpr = np.asarray(inp["last_pos_rel"], f32)
    x0 = lpr @ np.asarray(inp["W_se"], f32) + np.asarray(inp["b_se"], f32)

    def tsh(a, d):  # [B, d] -> [NCORES, d, BC]
        return np.ascontiguousarray(
            np.asarray(a, f32).reshape(NCORES, BC, d).transpose(0, 2, 1))

    lp = tsh(inp["last_pos"], 2)                           # [NC, 2, BC]
    lp1 = np.concatenate([lp, np.ones((NCORES, 1, BC), f32)], 1)
    return {"h0": tsh(inp["hh"], HD), "c0": tsh(inp["ch"], HD),
            "lp0": lp1, "x0": tsh(x0, ED)}


# ---------------------------------------------------------------- bass build

def _build_nc():
    import concourse.bass as bass
    import concourse.mybir as mybir
    import concourse.tile as tile
    from concourse import bacc

    F32 = mybir.dt.float32
    BF16 = mybir.dt.bfloat16
    AF = mybir.ActivationFunctionType
    ALU = mybir.AluOpType
    AX = mybir.AxisListType

    nc = bacc.Bacc()
    EI, EO = "ExternalInput", "ExternalOutput"
    di = {}
    for name, shape, dt in [
        ("Cmat", [64, SC * 2 * P * P], BF16),
        ("invp8", [128, NBLK * P * P], F32),
        ("moi8", [128, NBLK * P * P], F32),
        ("wnorm8", [128, NBLK * P * P], F32),
        ("WihT", [ED, 4 * HD], F32), ("WhhT", [HD, 4 * HD], F32),
        ("bg", [4 * HD, 1], F32),
        ("Whp", [HD, 2], F32), ("bhp", [2, 1], F32),
        ("Wse", [2, ED], F32), ("bse", [ED, 1], F32),
        ("Wst", [35, 4 * GH], F32),
        ("W2p", [GH, 16 * 128], BF16),
        ("Wm1", [51, MD], F32), ("Wm2", [MD + 1, HD], F32),
        ("h0", [HD, BC], F32), ("c0", [HD, BC], F32),
        ("lp0", [3, BC], F32), ("x0", [ED, BC], F32),
    ]:
        di[name] = nc.dram_tensor(name, shape, dt, kind=EI)
    rels = nc.dram_tensor("rels", [T, 2, BC], F32, kind=EO)

    with tile.TileContext(nc) as tc:
        with tc.tile_pool(name="cst", bufs=1) as cst, \
             tc.tile_pool(name="wk", bufs=3) as wk, \
             tc.tile_pool(name="wkb", bufs=4) as wkb, \
             tc.tile_pool(name="dsc", bufs=2, space="DRAM") as dsc, \
             tc.tile_pool(name="pmisc", bufs=1, space="PSUM") as pmisc, \
             tc.tile_pool(name="ptu", bufs=2, space="PSUM") as ptu, \
             tc.tile_pool(name="ppre", bufs=3, space="PSUM") as ppre, \
             tc.tile_pool(name="pgt", bufs=1, space="PSUM") as pgt:

            t_ = {}
            for name, h in di.items():
                if name in ("h0", "c0", "lp0", "x0"):
                    continue
                sh, dt = list(h.shape), h.dtype
                t_[name] = cst.tile(sh, dt, tag=name, name=name + "_t")
                nc.sync.dma_start(t_[name][:], h[:])

            U = cst.tile([51, BC], F32, tag="U")   # h2(0:32) pos(32:34) one(34) ph(35:51)
            dhS = cst.tile([MD + 1, BC], F32, tag="dhS")
            hT = cst.tile([HD, BC], F32, tag="hT")
            cT = cst.tile([HD, BC], F32, tag="cT")
            xT = cst.tile([ED, BC], F32, tag="xT")
            posT = cst.tile([2, BC], F32, tag="posT")
            nc.vector.memset(dhS[MD:MD + 1, :], 1.0)
            nc.sync.dma_start(U[32:35, :], di["lp0"][:])
            nc.sync.dma_start(posT[:], di["lp0"][0:2, :])
            nc.sync.dma_start(hT[:], di["h0"][:])
            nc.sync.dma_start(cT[:], di["c0"][:])
            nc.sync.dma_start(xT[:], di["x0"][:])

            for step in range(T):
                # ---- LSTM cell (transposed): gates [128, BC]
                gates = pmisc.tile([4 * HD, BC], F32, tag="pm")
                nc.tensor.matmul(gates[:], t_["WihT"][:], xT[:],
                                 start=True, stop=False)
                nc.tensor.matmul(gates[:], t_["WhhT"][:], hT[:],
                                 start=False, stop=True)
                si = wk.tile([HD, BC], F32, tag="si")
                sf = wk.tile([HD, BC], F32, tag="sf")
                tg = wk.tile([HD, BC], F32, tag="tg")
                so = wk.tile([HD, BC], F32, tag="so")
                nc.scalar.activation(si[:], gates[0:32], AF.Sigmoid,
                                     bias=t_["bg"][0:32])
                nc.scalar.activation(sf[:], gates[32:64], AF.Sigmoid,
                                     bias=t_["bg"][32:64])
                nc.scalar.activation(tg[:], gates[64:96], AF.Tanh,
                                     bias=t_["bg"][64:96])
                nc.scalar.activation(so[:], gates[96:128], AF.Sigmoid,
                                     bias=t_["bg"][96:128])
                tmp = wk.tile([HD, BC], F32, tag="tmp")
                nc.vector.tensor_mul(tmp[:], si[:], tg[:])
                nc.vector.tensor_mul(cT[:], sf[:], cT[:])
                nc.vector.tensor_add(cT[:], cT[:], tmp[:])
                tc2 = wk.tile([HD, BC], F32, tag="tc2")
                nc.scalar.activation(tc2[:], cT[:], AF.Tanh)
                nc.vector.tensor_mul(U[0:32, :], so[:], tc2[:])

                # ---- rel_pos, cur, next x
                rp_p = pmisc.tile([2, BC], F32, tag="pm")
                nc.tensor.matmul(rp_p[:], t_["Whp"][:], U[0:32, :],
                                 start=True, stop=True)
                rpT = wk.tile([2, BC], F32, tag="rpT")
                nc.scalar.activation(rpT[:], rp_p[:], AF.Identity,
                                     bias=t_["bhp"][:])
                nc.sync.dma_start(rels[step], rpT[:])
                nc.vector.tensor_add(posT[:], posT[:], rpT[:])
                nc.scalar.copy(U[32:34, :], posT[:])
                xp = pmisc.tile([ED, BC], F32, tag="pm")
                nc.tensor.matmul(xp[:], t_["Wse"][:], rpT[:],
                                 start=True, stop=True)
                nc.scalar.activation(xT[:], xp[:], AF.Identity, bias=t_["bse"][:])

                # ---- pooling
                for b in range(NBLK):
                    gtp = pgt.tile([128, P * P], F32, tag="gtp")
                    for ls in range(8):
                        sl = b * 8 + ls
                        pcol = slice(sl * P, (sl + 1) * P)
                        for m in range(2):
                            g = ls * 2 + m
                            tup_t = ptu.tile([32, GH], F32, tag="tup")
                            nc.tensor.matmul(tup_t[:],
                                             U[0:35, pcol],
                                             t_["Wst"][:, m * 72:(m + 1) * 72],
                                             start=True, stop=True)
                            tup_u = ptu.tile([32, GH], F32, tag="tup")
                            nc.tensor.matmul(tup_u[:],
                                             U[0:35, pcol],
                                             t_["Wst"][:, 144 + m * 72:144 + (m + 1) * 72],
                                             start=True, stop=True)
                            tus = wkb.tile([64, GH], BF16, tag="tus")
                            nc.scalar.copy(tus[0:32, :], tup_t[:])
                            nc.scalar.copy(tus[32:64, :], tup_u[:])
                            ccol = (sl * 2 + m) * P * P
                            for c in range(2):
                                prep = ppre.tile([GH, 512], F32, tag="prep")
                                nc.tensor.matmul(
                                    prep[:], tus[:],
                                    t_["Cmat"][:, ccol + c * 512:ccol + (c + 1) * 512],
                                    start=True, stop=True)
                                h1 = wkb.tile([GH, 512], BF16, tag="h1")
                                if m == 0:
                                    nc.scalar.activation(h1[:], prep[:], AF.Relu)
                                else:
                                    nc.vector.tensor_relu(h1[:], prep[:])
                                nc.tensor.matmul(
                                    gtp[:, c * 512:(c + 1) * 512],
                                    t_["W2p"][:, g * 128:(g + 1) * 128], h1[:],
                                    start=(g == 0), stop=(g == 15))
                    # ---- aggregation block: 16 groups x [8, 1024]
                    bcol = slice(b * P * P, (b + 1) * P * P)
                    gs = wk.tile([128, P * P], F32, tag="gs")
                    nc.vector.tensor_copy(gs[:], gtp[:])
                    gd = wk.tile([128, P], F32, tag="gd")
                    nc.vector.tensor_copy(gd[:], gs[:, 0:P * P:P + 1])
                    o2 = wk.tile([128, P * P], F32, tag="o2")
                    nc.vector.tensor_mul(o2[:], gs[:], t_["invp8"][:, bcol])
                    tmp2 = wk.tile([128, P * P], F32, tag="tmp2")
                    nc.vector.tensor_mul(
                        tmp2[:].rearrange("p (i j) -> p i j", i=P),
                        t_["moi8"][:, bcol].rearrange("p (i j) -> p i j", i=P),
                        gd[:].broadcast_to((128, P, P)))
                    nc.vector.tensor_add(o2[:], o2[:], tmp2[:])
                    nc.vector.tensor_mul(tmp2[:], gs[:], t_["wnorm8"][:, bcol])
                    dv = wk.tile([128, P], F32, tag="dv")
                    nc.vector.tensor_reduce(
                        dv[:], tmp2[:].rearrange("p (i j) -> p i j", i=P),
                        axis=AX.X, op=ALU.add)
                    nc.scalar.activation(o2[:], o2[:], AF.Relu)
                    nc.vector.tensor_relu(dv[:], dv[:])
                    nc.vector.tensor_copy(o2[:, 0:P * P:P + 1], dv[:])
                    mx = wk.tile([128, P], F32, tag="mx")
                    nc.vector.tensor_reduce(
                        mx[:], o2[:].rearrange("p (i j) -> p i j", i=P),
                        axis=AX.X, op=ALU.max)
                    phs = dsc.tile([128, P], F32, tag="phs")
                    nc.sync.dma_start(phs[:], mx[:])
                    nc.sync.dma_start(
                        U[35:51, b * 256:(b + 1) * 256],
                        phs[:].rearrange("(ls m o) i -> (m o) ls i", ls=8, m=2))

                # ---- MLP -> next h
                mh = pmisc.tile([MD, BC], F32, tag="pm")
                nc.tensor.matmul(mh[:], t_["Wm1"][:], U[0:51, :],
                                 start=True, stop=True)
                nc.scalar.activation(dhS[0:MD, :], mh[:], AF.Relu)
                hp2 = pmisc.tile([HD, BC], F32, tag="pm")
                nc.tensor.matmul(hp2[:], t_["Wm2"][:], dhS[:],
                                 start=True, stop=True)
                nc.scalar.activation(hT[:], hp2[:], AF.Relu)
    nc.finalize()
    return nc


# ---------------------------------------------------------------- jit runner

def _get_runtime():
    global _RT
    if _RT is not None:
        return _RT
    import jax
    import concourse.mybir as mybir
    from concourse import bass2jax
    from jax.sharding import Mesh, PartitionSpec
    from jax.experimental.shard_map import shard_map

    bass2jax.install_neuronx_cc_hook()
    nc = _build_nc()

    pname = nc.partition_id_tensor.name if nc.partition_id_tensor else None
    in_names, out_names, out_avals = [], [], []
    for alloc in nc.m.functions[0].allocations:
        if not isinstance(alloc, mybir.MemoryLocationSet):
            continue
        name = alloc.memorylocations[0].name
        if alloc.kind == "ExternalInput":
            if name != pname:
                in_names.append(name)
        elif alloc.kind == "ExternalOutput":
            out_names.append(name)
            out_avals.append(jax.core.ShapedArray(
                tuple(alloc.tensor_shape), mybir.dt.np(alloc.dtype)))
    n_params = len(in_names)
    all_names = in_names + out_names
    if pname is not None:
        all_names = all_names + [pname]
    donate = tuple(range(n_params, n_params + len(out_names)))

    def _body(*args):
        operands = list(args)
        if pname is not None:
            operands.append(bass2jax.partition_id_tensor())
        return tuple(bass2jax._bass_exec_p.bind(
            *operands, out_avals=tuple(out_avals), in_names=tuple(all_names),
            out_names=tuple(out_names), lowering_input_output_aliases=(),
            sim_require_finite=True, sim_require_nnan=True, nc=nc))

    mesh = Mesh(np.asarray(jax.devices()[:NCORES]), ("core",))
    spec = (PartitionSpec("core"),) * (n_params + len(out_names))
    sharded = jax.jit(
        shard_map(_body, mesh=mesh, in_specs=spec,
                  out_specs=(PartitionSpec("core"),) * len(out_names),
                  check_rep=False),
        donate_argnums=donate, keep_unused=True)
    out_shapes = [tuple(a.shape) for a in out_avals]
    _RT = (sharded, in_names, n_params, out_names, out_shapes, mesh)
    return _RT


def _consts_key(inp):
    h = hashlib.md5()
    for k in ("end_group", "W_se", "b_se", "Wih", "Whh", "bih", "bhh",
              "W_hp", "b_hp", "W_pse", "b_pse", "W1a", "W2a", "W1b", "W2b",
              "W_m1", "b_m1", "W_m2", "b_m2"):
        h.update(np.ascontiguousarray(inp[k]).tobytes())
    return h.digest()


def kernel(**inputs):
    global _CONSTS
    import jax
    from jax.sharding import NamedSharding, PartitionSpec

    sharded, in_names, n_params, out_names, out_shapes, mesh = _get_runtime()
    shard = NamedSharding(mesh, PartitionSpec("core"))

    key = _consts_key(inputs)
    if _CONSTS is None or _CONSTS[0] != key:
        cn = _prep_consts(inputs)
        dev = {k: jax.device_put(
            np.ascontiguousarray(v.reshape(-1, *v.shape[2:])), shard)
            for k, v in cn.items()}
        _CONSTS = (key, dev)
    consts = _CONSTS[1]

    st = _prep_state(inputs)
    args = []
    for name in in_names:
        if name in consts:
            args.append(consts[name])
        else:
            v = st[name]
            args.append(v.reshape(-1, *v.shape[2:]))
    for shp in out_shapes:
        args.append(np.zeros((NCORES * shp[0],) + shp[1:], np.float32))

    outs = sharded(*args)
    rels = np.asarray(outs[out_names.index("rels")])
    rels = rels.reshape(NCORES, T, 2, BC).transpose(1, 0, 3, 2)
    return np.ascontiguousarray(rels.reshape(T, B, 2))
